# revision 17
# baseline (speedup 1.0000x reference)
"""Trainium2 Bass kernel for the 3-layer GAT + MLP DQN (nn_DQN_86655260164180).

Distribution (8 NeuronCores, SPMD):
  - Nodes sharded contiguously: core c owns nodes [c*6250,(c+1)*6250), padded
    to 6272 = 49*128. Edges (incl. self-loops) assigned to the dst's core,
    sorted by dst; each 128-dst block's edge run is padded to a multiple of
    128 ("chunks"), uniform K_max chunks/block across cores (one SPMD NEFF).
  - Per GAT layer: per-edge logits from two cheap indirect gathers (ls[src],
    ld[dst], 16B rows); ex = exp(leaky_relu(ls+ld)) batched over all edges
    (exp-max identity removes the segment-max exactly). Pad edges point ld at
    a -1e30 row so ex == 0.
  - Message sum in S-form: out[d] = sum_h (sum_e ex*x_src) @ W_h/(H*den)
    so only x[src] (C_in floats) is gathered per edge; the scatter is a
    one-hot matmul S_psum += M0^T @ (ex (x) x), M0 from a DVE eq vs iota.
  - BN stats exact via per-shard sums + tiny AllReduce; next-layer tables
    rebuilt on the owning shard (BN affine+ELU+dropout mask from the fixed
    jax key-42 masks, host-precomputed), transposed to rows, AllGathered.
  - Final per-node MLP (+LN, tanh) per 128-node block; softmax on host.
"""
import numpy as np

N = 50000
E = 400000
H = 4
NCORES = 8
SHARD = 6250
SPAD = 6272           # 49 * 128
NBLK = 49
P = 128
NPAD = NCORES * SPAD  # 50176
EPS = 1e-5
NEG = -1e30
LRELU_ALPHA = 0.2
LAYERS = [(4, 128), (128, 64), (64, 32)]
# padded table row lengths (f32 elems, 256B multiples) per layer: [x|ls|pad]
ROWP = [64, 192, 128]
LDROW = 64
GI = 1024            # indices per dma_gather instruction
IBASE = 32768        # int16 sign-offset base for src-index gathers
DEBUG_PROBES = False
TRUNC = 0

_CACHE = {}


def _prep_graph(edge_index):
    src = np.asarray(edge_index[0]).astype(np.int64)
    dst = np.asarray(edge_index[1]).astype(np.int64)
    loops = np.arange(N, dtype=np.int64)
    src = np.concatenate([src, loops])
    dst = np.concatenate([dst, loops])
    core_of = dst // SHARD
    per_core = []
    K_max = 1
    for c in range(NCORES):
        m = core_of == c
        s = src[m]
        dl = dst[m] - c * SHARD
        order = np.argsort(dl, kind="stable")
        s, dl = s[order], dl[order]
        counts = np.bincount(dl // P, minlength=NBLK)
        K_max = max(K_max, int(np.ceil(counts.max() / P)))
        per_core.append((s, dl, counts))
    ncols = NBLK * K_max
    epad = ncols * P
    out = []
    for c in range(NCORES):
        s, dl, counts = per_core[c]
        src_g = np.zeros(epad, np.int64)
        dst_gl = np.full(epad, SPAD, np.int64)   # pad edges -> NEG row
        dst_loc = np.zeros(epad, np.int64)
        pos = 0
        for b in range(NBLK):
            cnt = int(counts[b])
            w = b * K_max * P
            seg_s = s[pos:pos + cnt]
            seg_d = dl[pos:pos + cnt]
            src_g[w:w + cnt] = (seg_s // SHARD) * SPAD + (seg_s % SHARD)
            dst_gl[w:w + cnt] = seg_d
            dst_loc[w:w + cnt] = seg_d - b * P
            pos += cnt
        out.append((
            src_g.reshape(ncols, P).T.astype(np.int32).copy(),
            dst_gl.reshape(ncols, P).T.astype(np.int32).copy(),
            dst_loc.reshape(ncols, P).T.astype(np.float32).copy(),
        ))
    return out, K_max, ncols


def _dropout_masks():
    import jax
    with jax.default_device(jax.devices("cpu")[0]):
        dk = jax.random.split(jax.random.key(42), 3)
        masks = []
        for k, c in zip(dk, (128, 64, 32)):
            keep = jax.random.bernoulli(k, 0.5, (N, c))
            masks.append(np.asarray(jax.device_get(keep)).astype(np.float32) * 2.0)
    return masks


def _build_nc(K_max, ncols):
    import concourse.bacc as bacc
    import concourse.bass as bass
    import concourse.tile as tile
    from concourse import mybir
    from concourse.masks import make_identity
    from contextlib import ExitStack

    f32 = mybir.dt.float32
    i32 = mybir.dt.int32
    AF = mybir.ActivationFunctionType
    OP = mybir.AluOpType
    RG = [list(range(NCORES))]

    nc = bacc.Bacc("TRN2", target_bir_lowering=False, debug=False,
                   num_devices=NCORES)

    i16 = mybir.dt.int16
    NW = (ncols * P) // 16   # wrapped idx columns
    t_l1 = nc.dram_tensor("t_l1", [NPAD, ROWP[0]], f32, kind="ExternalInput").ap()
    ld1 = nc.dram_tensor("ld1", [SPAD + P, LDROW], f32, kind="ExternalInput").ap()
    src_gA = nc.dram_tensor("src_gA", [P, NW], i16, kind="ExternalInput").ap()
    src_gB = nc.dram_tensor("src_gB", [P, NW], i16, kind="ExternalInput").ap()
    selB_in = nc.dram_tensor("selB", [P, ncols], mybir.dt.uint8,
                             kind="ExternalInput").ap()
    dst_gl = nc.dram_tensor("dst_gl", [P, NW], i16, kind="ExternalInput").ap()
    dst_loc = nc.dram_tensor("dst_loc", [P, ncols], f32, kind="ExternalInput").ap()
    iota_in = nc.dram_tensor("iota", [P, P], f32, kind="ExternalInput").ap()
    win = {}

    def din(name, shape):
        win[name] = nc.dram_tensor(name, shape, f32, kind="ExternalInput").ap()

    for l, (ci, co) in enumerate(LAYERS, start=1):
        din(f"wstack{l}", [H * ci, co])
        din(f"g{l}", [co, 1])
        din(f"be{l}", [co, 1])
    din("ws2", [128, 4]); din("wd2", [128, 4])
    din("ws3", [64, 4]); din("wd3", [64, 4])
    din("a1e", [33, 128]); din("a2", [128, 64]); din("a3", [64, 1])
    din("ab2r", [1, 64]); din("ab3", [1, 1])
    din("l1g", [1, 128]); din("l1b", [1, 128])
    din("l2g", [1, 64]); din("l2b", [1, 64])
    mask_in = [nc.dram_tensor(f"mask{l}", [co, SPAD], f32, kind="ExternalInput").ap()
               for l, (ci, co) in enumerate(LAYERS, start=1)]
    tout = nc.dram_tensor("tout", [SPAD, 1], f32, kind="ExternalOutput").ap()
    probes = {}
    if DEBUG_PROBES:
        for pname, shape in (("p_ls", [P, ncols * 4]), ("p_ld", [P, ncols * 4]),
                             ("p_ex", [P, ncols * 4]), ("p_xo", [128, SPAD]),
                             ("p_xn", [128, SPAD])):
            probes[pname] = nc.dram_tensor(pname, shape, f32,
                                           kind="ExternalOutput").ap()

    with tile.TileContext(nc) as tc, ExitStack() as ctx:
        consts = ctx.enter_context(tc.tile_pool(name="consts", bufs=1))
        edges = ctx.enter_context(tc.tile_pool(name="edges", bufs=1))
        xg_pool = ctx.enter_context(tc.tile_pool(name="xg", bufs=3))
        zp = ctx.enter_context(tc.tile_pool(name="zp", bufs=3))
        m0p = ctx.enter_context(tc.tile_pool(name="m0p", bufs=3))
        sev = ctx.enter_context(tc.tile_pool(name="sev", bufs=3))
        feat = ctx.enter_context(tc.tile_pool(name="feat", bufs=1))
        small = ctx.enter_context(tc.tile_pool(name="small", bufs=4))
        rowp = ctx.enter_context(tc.tile_pool(name="rowp", bufs=3))
        psS = ctx.enter_context(tc.tile_pool(name="psS", bufs=2, space="PSUM"))
        psD = ctx.enter_context(tc.tile_pool(name="psD", bufs=2, space="PSUM"))
        psM = ctx.enter_context(tc.tile_pool(name="psM", bufs=4, space="PSUM"))
        dram = ctx.enter_context(tc.tile_pool(name="dram", bufs=1, space="DRAM"))

        # ---------------- constants ----------------
        ident = consts.tile([P, P], f32)
        make_identity(nc, ident[:])
        iota = consts.tile([P, P], f32)
        nc.sync.dma_start(out=iota[:], in_=iota_in[:])
        idx_srcA = consts.tile([P, NW], i16)
        nc.sync.dma_start(out=idx_srcA[:], in_=src_gA[:])
        idx_srcB = consts.tile([P, NW], i16)
        nc.sync.dma_start(out=idx_srcB[:], in_=src_gB[:])
        selB = consts.tile([P, ncols], mybir.dt.uint8)
        nc.sync.dma_start(out=selB[:], in_=selB_in[:])
        idx_dst = consts.tile([P, NW], i16)
        nc.sync.dma_start(out=idx_dst[:], in_=dst_gl[:])
        dcol = consts.tile([P, ncols], f32)
        nc.sync.dma_start(out=dcol[:], in_=dst_loc[:])
        wsb = {}
        for name, ap_ in win.items():
            if ap_.shape[0] > P:
                slices = []
                for s0 in range(0, ap_.shape[0], P):
                    s1 = min(s0 + P, ap_.shape[0])
                    t = consts.tile([s1 - s0, ap_.shape[1]], f32,
                                    tag=f"{name}_{s0}")
                    nc.sync.dma_start(out=t[:], in_=ap_[s0:s1, :])
                    slices.append(t)
                wsb[name] = slices
                continue
            t = consts.tile(list(ap_.shape), f32, tag=name)
            nc.sync.dma_start(out=t[:], in_=ap_[:])
            wsb[name] = t
        bcast = {}
        for name, width in (("l1g", 128), ("l1b", 128), ("l2g", 64),
                            ("l2b", 64), ("ab2r", 64)):
            bt = consts.tile([P, width], f32, tag="bc" + name)
            nc.gpsimd.partition_broadcast(bt[:], wsb[name][:])
            bcast[name] = bt
        ab3b = consts.tile([P, 1], f32)
        nc.gpsimd.partition_broadcast(ab3b[:], wsb["ab3"][:])
        negrow = consts.tile([1, LDROW], f32)
        nc.vector.memset(negrow[:], NEG)

        NE4 = ncols * 4
        KSC = GI // P            # chunks per gather instruction (8)
        cur_tab, cur_ld, cur_row = t_l1, ld1, ROWP[0]
        h3 = None

        for l, (ci, co) in enumerate(LAYERS, start=1):
            last = l == 3
            rp = cur_row
            # ---------- ld gathers (compact into resident ld_e) ----------
            ld_e = edges.tile([P, NE4], f32, tag="ld")
            for j0 in range(0, ncols, KSC):
                j1 = min(j0 + KSC, ncols)
                ni = (j1 - j0) * P
                lg = zp.tile([P, KSC * LDROW], f32, tag="ldg")
                nc.gpsimd.dma_gather(
                    out_ap=lg[:, :(j1 - j0) * LDROW].rearrange(
                        "p (k r) -> p k r", r=LDROW),
                    in_ap=cur_ld[:, :], idxs_ap=idx_dst[:, j0 * 8:j1 * 8],
                    num_idxs=ni, num_idxs_reg=ni, elem_size=LDROW)
                nc.vector.tensor_copy(
                    ld_e[:, j0 * 4:j1 * 4].rearrange("p (k f) -> p k f", f=4),
                    lg[:, :(j1 - j0) * LDROW].rearrange(
                        "p (k r) -> p k r", r=LDROW)[:, :, 0:4])
            ex_e = edges.tile([P, NE4], f32, tag="ex")

            # xout: feat-major [co(+1), SPAD]; becomes x_next in place
            xo = feat.tile([33 if last else co, SPAD], f32, tag="xo")

            # ---------- x(+ls) gathers + per-tile ex pipeline ----------
            xg_tiles = []
            for j0 in range(0, ncols, KSC):
                j1 = min(j0 + KSC, ncols)
                ni = (j1 - j0) * P
                xg = xg_pool.tile([P, KSC * rp], f32, tag="xg")
                xgv = xg[:, :(j1 - j0) * rp].rearrange("p (k r) -> p k r", r=rp)
                nc.gpsimd.dma_gather(
                    out_ap=xgv,
                    in_ap=cur_tab[0:IBASE, :],
                    idxs_ap=idx_srcA[:, j0 * 8:j1 * 8],
                    num_idxs=ni, num_idxs_reg=ni, elem_size=rp)
                xh = xg_pool.tile([P, KSC * rp], f32, tag="xh")
                xhv = xh[:, :(j1 - j0) * rp].rearrange("p (k r) -> p k r", r=rp)
                nc.gpsimd.dma_gather(
                    out_ap=xhv,
                    in_ap=cur_tab[IBASE:NPAD, :],
                    idxs_ap=idx_srcB[:, j0 * 8:j1 * 8],
                    num_idxs=ni, num_idxs_reg=ni, elem_size=rp)
                nc.vector.copy_predicated(
                    out=xgv, data=xhv,
                    mask=selB[:, j0:j1].unsqueeze(2).to_broadcast(
                        [P, j1 - j0, rp]))
                # ex for these chunks: exp(lrelu(ls_src + ld_dst))
                exs = ex_e[:, j0 * 4:j1 * 4]
                nc.vector.tensor_tensor(
                    out=exs.rearrange("p (k f) -> p k f", f=4),
                    in0=xgv[:, :, ci:ci + 4],
                    in1=ld_e[:, j0 * 4:j1 * 4].rearrange("p (k f) -> p k f", f=4),
                    op=OP.add)
                lrt = zp.tile([P, KSC * 4], f32, tag="lrt")
                nc.vector.tensor_scalar(out=lrt[:, :(j1 - j0) * 4], in0=exs,
                                        scalar1=LRELU_ALPHA, scalar2=None,
                                        op0=OP.mult)
                nc.vector.tensor_tensor(out=exs, in0=exs,
                                        in1=lrt[:, :(j1 - j0) * 4], op=OP.max)
                nc.scalar.activation(out=exs, in_=exs, func=AF.Exp)
                xg_tiles.append((j0, j1, xg))
            if TRUNC == 2 and l == 1:
                nc.sync.dma_start(out=tout[0:P, :], in_=ex_e[:, 0:1])
                h3 = None
                break
            if DEBUG_PROBES and l == 1:
                nc.sync.dma_start(out=probes["p_ld"][:], in_=ld_e[:])
                nc.sync.dma_start(out=probes["p_ex"][:], in_=ex_e[:])

            # ---------- edge phase ----------
            for b in range(NBLK):
                s_ps = psS.tile([P, H * ci], f32, tag="S", space="PSUM")
                d_ps = psD.tile([P, 4], f32, tag="den", space="PSUM")
                for k in range(K_max):
                    c = b * K_max + k
                    for (j0, j1, xg) in xg_tiles:
                        if j0 <= c < j1:
                            break
                    xsl = xg[:, (c - j0) * rp:(c - j0) * rp + ci]
                    exsl = ex_e[:, c * 4:(c + 1) * 4]
                    m0 = m0p.tile([P, P], f32, tag="m0")
                    nc.vector.tensor_scalar(
                        out=m0[:], in0=iota[:], scalar1=dcol[:, c:c + 1],
                        scalar2=None, op0=OP.is_equal)
                    z = zp.tile([P, H, ci], f32, tag="z")
                    nc.vector.tensor_tensor(
                        out=z[:],
                        in0=xsl.unsqueeze(1).to_broadcast([P, H, ci]),
                        in1=exsl.unsqueeze(2).to_broadcast([P, H, ci]),
                        op=OP.mult)
                    nc.tensor.matmul(out=s_ps[:], lhsT=m0[:],
                                     rhs=z[:].rearrange("p h c -> p (h c)"),
                                     start=(k == 0), stop=(k == K_max - 1))
                    nc.tensor.matmul(out=d_ps[:], lhsT=m0[:], rhs=exsl,
                                     start=(k == 0), stop=(k == K_max - 1))

                # ---------- block epilogue: S/(den) -> dense -> xo ----------
                rden = small.tile([P, 4], f32, tag="rden")
                nc.vector.reciprocal(out=rden[:], in_=d_ps[:])
                s_sb = sev.tile([P, H * ci], f32, tag="s_sb")
                nc.vector.tensor_tensor(
                    out=s_sb[:].rearrange("p (h c) -> p h c", h=H),
                    in0=s_ps[:].rearrange("p (h c) -> p h c", h=H),
                    in1=rden[:].unsqueeze(2).to_broadcast([P, H, ci]),
                    op=OP.mult)
                o_ps = psM.tile([co, P], f32, tag="m", space="PSUM")
                nslice = (H * ci + P - 1) // P
                wstk = wsb[f"wstack{l}"]
                if not isinstance(wstk, list):
                    wstk = [wstk]
                for s in range(nslice):
                    w0 = s * P
                    w1 = min(w0 + P, H * ci)
                    w = w1 - w0
                    st_ps = psM.tile([P, P], f32, tag="m", space="PSUM")
                    nc.tensor.transpose(out=st_ps[:w, :], in_=s_sb[:, w0:w1],
                                        identity=ident[:])
                    st_sb = sev.tile([P, P], f32, tag="st_sb")
                    nc.scalar.copy(out=st_sb[:w, :], in_=st_ps[:w, :])
                    nc.tensor.matmul(out=o_ps[:], lhsT=wstk[s][:w, :],
                                     rhs=st_sb[:w, :],
                                     start=(s == 0), stop=(s == nslice - 1))
                ncol_b = 106 if b == NBLK - 1 else P
                nc.scalar.copy(out=xo[:co, b * P:b * P + ncol_b],
                               in_=o_ps[:, :ncol_b])
            nc.vector.memset(xo[:co, SHARD:SPAD], 0)
            if DEBUG_PROBES and l == 1:
                nc.sync.dma_start(out=probes["p_xo"][:co, :], in_=xo[:co, :])
            if TRUNC == 1 and l == 1:
                nc.sync.dma_start(out=tout[0:P, :], in_=xo[:co, 0:1])
                h3 = None
                break

            # ---------- BN stats + AllReduce ----------
            scratch = feat.tile([128, SPAD], f32, tag="scratch")
            stat = small.tile([co, 2], f32, tag="stat")
            nc.vector.tensor_reduce(out=stat[:, 0:1], in_=xo[:co, :],
                                    axis=mybir.AxisListType.XYZW, op=OP.add)
            nc.scalar.square(out=scratch[:co, :], in_=xo[:co, :])
            nc.vector.tensor_reduce(out=stat[:, 1:2], in_=scratch[:co, :],
                                    axis=mybir.AxisListType.XYZW, op=OP.add)
            st_in = dram.tile([co, 2], f32, tag=f"sti{l}")
            st_out = dram.tile([co, 2], f32, tag=f"sto{l}")
            nc.gpsimd.dma_start(out=st_in[:], in_=stat[:])
            nc.gpsimd.collective_compute("AllReduce", OP.add, replica_groups=RG,
                                         ins=[st_in.opt()], outs=[st_out.opt()])
            gstat = small.tile([co, 2], f32, tag="gstat")
            nc.sync.dma_start(out=gstat[:], in_=st_out[:])
            mu = small.tile([co, 1], f32, tag="mu")
            nc.scalar.mul(out=mu[:], in_=gstat[:, 0:1], mul=1.0 / N)
            var = small.tile([co, 1], f32, tag="var")
            nc.scalar.mul(out=var[:], in_=gstat[:, 1:2], mul=1.0 / N)
            musq = small.tile([co, 1], f32, tag="musq")
            nc.vector.tensor_mul(musq[:], mu[:], mu[:])
            nc.vector.tensor_sub(var[:], var[:], musq[:])
            rst = small.tile([co, 1], f32, tag="rst")
            nc.vector.tensor_scalar(out=var[:], in0=var[:], scalar1=EPS,
                                    scalar2=None, op0=OP.add)
            nc.vector.reciprocal(out=rst[:], in_=var[:])
            nc.scalar.sqrt(out=rst[:], in_=rst[:])
            aa = small.tile([co, 1], f32, tag="aa")
            nc.vector.tensor_mul(aa[:], rst[:], wsb[f"g{l}"][:])
            bb = small.tile([co, 1], f32, tag="bb")
            nc.vector.tensor_mul(bb[:], mu[:], aa[:])
            nc.vector.tensor_scalar(out=bb[:], in0=bb[:], scalar1=-1.0,
                                    scalar2=None, op0=OP.mult)
            nc.vector.tensor_add(bb[:], bb[:], wsb[f"be{l}"][:])

            # ---------- post-process in place: xo <- mask*act(aa*xo+bb) ----------
            maskt = feat.tile([co, SPAD], f32, tag="maskt")
            nc.sync.dma_start(out=maskt[:], in_=mask_in[l - 1][:])
            if not last:
                # elu(t) = relu(t) + min(exp(t),1)-1
                nc.scalar.activation(out=scratch[:co, :], in_=xo[:co, :],
                                     func=AF.Exp, bias=bb[:], scale=aa[:])
                nc.vector.tensor_scalar(out=scratch[:co, :], in0=scratch[:co, :],
                                        scalar1=1.0, scalar2=-1.0,
                                        op0=OP.min, op1=OP.add)
                nc.scalar.activation(out=xo[:co, :], in_=xo[:co, :],
                                     func=AF.Relu, bias=bb[:], scale=aa[:])
                nc.vector.tensor_add(xo[:co, :], xo[:co, :], scratch[:co, :])
            else:
                nc.vector.tensor_scalar(out=xo[:co, :], in0=xo[:co, :],
                                        scalar1=aa[:], scalar2=bb[:],
                                        op0=OP.mult, op1=OP.add)
            nc.vector.tensor_mul(xo[:co, :], xo[:co, :], maskt[:])
            if DEBUG_PROBES and l == 1:
                nc.sync.dma_start(out=probes["p_xn"][:co, :], in_=xo[:co, :])
            if last:
                nc.vector.memset(xo[co:co + 1, :], 1.0)
                h3 = xo
                break

            # ---------- next tables (rows) + AllGather ----------
            rowlen = ROWP[l]
            tab_in = dram.tile([SPAD, rowlen], f32, tag=f"tabin{l}")
            tab_out = dram.tile([NPAD, rowlen], f32, tag=f"tabout{l}")
            ld_tab = dram.tile([SPAD + P, LDROW], f32, tag=f"ldtab{l}")
            for b in range(NBLK):
                cols = slice(b * P, (b + 1) * P)
                rows = rowp.tile([P, rowlen + 4], f32, tag="rows")
                tp = psM.tile([P, P], f32, tag="m", space="PSUM")
                nc.tensor.transpose(out=tp[:, :co], in_=xo[:co, cols],
                                    identity=ident[:co, :co])
                nc.scalar.copy(out=rows[:, :co], in_=tp[:, :co])
                for wname, colr in ((f"ws{l + 1}", slice(co, co + 4)),
                                    (f"wd{l + 1}", slice(co + 4, co + 8))):
                    lp = psM.tile([4, P], f32, tag="m", space="PSUM")
                    nc.tensor.matmul(out=lp[:], lhsT=wsb[wname][:],
                                     rhs=xo[:co, cols], start=True, stop=True)
                    lsb = small.tile([4, P], f32, tag="lsb")
                    nc.scalar.copy(out=lsb[:], in_=lp[:])
                    lt = psM.tile([P, 4], f32, tag="m", space="PSUM")
                    nc.tensor.transpose(out=lt[:], in_=lsb[:],
                                        identity=ident[:4, :4])
                    nc.scalar.copy(out=rows[:, colr], in_=lt[:])
                nc.sync.dma_start(out=tab_in[cols, :co + 4],
                                  in_=rows[:, :co + 4])
                nc.sync.dma_start(out=ld_tab[cols, 0:4], in_=rows[:, co + 4:co + 8])
            nc.sync.dma_start(out=ld_tab[SPAD:SPAD + 1, :], in_=negrow[:])
            nc.gpsimd.collective_compute("AllGather", OP.bypass, replica_groups=RG,
                                         ins=[tab_in.opt()], outs=[tab_out.opt()])
            cur_tab, cur_ld, cur_row = tab_out[:], ld_tab[:], rowlen

        # ---------------- MLP head ----------------
        def ln_affine_relu(dst, src, width, gname, bname, do_relu):
            stats = small.tile([P, 6], f32, tag="lnst")
            nc.vector.bn_stats(out=stats[:], in_=src)
            mv = small.tile([P, 2], f32, tag="lnmv")
            nc.vector.bn_aggr(out=mv[:], in_=stats[:])
            varn = small.tile([P, 1], f32, tag="lnvar")
            nc.vector.tensor_scalar(out=varn[:], in0=mv[:, 1:2], scalar1=EPS,
                                    scalar2=None, op0=OP.add)
            rstd = small.tile([P, 1], f32, tag="lnrstd")
            nc.vector.reciprocal(out=rstd[:], in_=varn[:])
            nc.scalar.sqrt(out=rstd[:], in_=rstd[:])
            nb = small.tile([P, 1], f32, tag="lnnb")
            nc.vector.tensor_mul(nb[:], mv[:, 0:1], rstd[:])
            nc.vector.tensor_scalar(out=nb[:], in0=nb[:], scalar1=-1.0,
                                    scalar2=None, op0=OP.mult)
            nc.vector.tensor_scalar(out=dst, in0=src, scalar1=rstd[:],
                                    scalar2=nb[:], op0=OP.mult, op1=OP.add)
            nc.vector.tensor_mul(dst, dst, bcast[gname][:, :width])
            nc.vector.tensor_add(dst, dst, bcast[bname][:, :width])
            if do_relu:
                nc.scalar.activation(out=dst, in_=dst, func=AF.Relu)

        for b in range(NBLK if h3 is not None else 0):
            cols = slice(b * P, (b + 1) * P)
            p1 = psM.tile([P, P], f32, tag="m", space="PSUM")
            nc.tensor.matmul(out=p1[:], lhsT=h3[:, cols], rhs=wsb["a1e"][:],
                             start=True, stop=True)
            a1 = sev.tile([P, P], f32, tag="a1")
            ln_affine_relu(a1[:], p1[:], 128, "l1g", "l1b", True)
            t1 = psM.tile([P, P], f32, tag="m", space="PSUM")
            nc.tensor.transpose(out=t1[:], in_=a1[:], identity=ident[:])
            a1t = sev.tile([P, P], f32, tag="a1t")
            nc.scalar.copy(out=a1t[:], in_=t1[:])
            p2 = psM.tile([P, 64], f32, tag="m", space="PSUM")
            nc.tensor.matmul(out=p2[:], lhsT=a1t[:], rhs=wsb["a2"][:],
                             start=True, stop=True)
            a2 = sev.tile([P, 64], f32, tag="a2")
            nc.vector.tensor_add(a2[:], p2[:], bcast["ab2r"][:])
            ln_affine_relu(a2[:], a2[:], 64, "l2g", "l2b", True)
            t2 = psM.tile([64, P], f32, tag="m", space="PSUM")
            nc.tensor.transpose(out=t2[:], in_=a2[:], identity=ident[:])
            a2t = sev.tile([64, P], f32, tag="a2t")
            nc.scalar.copy(out=a2t[:], in_=t2[:])
            p3 = psM.tile([P, 1], f32, tag="m", space="PSUM")
            nc.tensor.matmul(out=p3[:], lhsT=a2t[:], rhs=wsb["a3"][:],
                             start=True, stop=True)
            tt = sev.tile([P, 1], f32, tag="tt")
            nc.scalar.activation(out=tt[:], in_=p3[:], func=AF.Tanh,
                                 bias=ab3b[:], scale=1.0)
            nc.sync.dma_start(out=tout[cols, :], in_=tt[:])

    nc.compile()
    return nc


def _host_inputs(inputs, graphs):
    x = np.asarray(inputs["x"], np.float32)
    xin = x[:, 1:5]
    ws = {}
    for l in (1, 2, 3):
        W = np.asarray(inputs[f"W{l}"], np.float32)
        a_s = np.asarray(inputs[f"as{l}"], np.float32)
        a_d = np.asarray(inputs[f"ad{l}"], np.float32)
        ci = W.shape[0]
        Wr = W.reshape(ci, H, W.shape[1] // H)
        ws[f"ws{l}"] = np.einsum("khc,hc->kh", Wr, a_s).astype(np.float32)
        ws[f"wd{l}"] = np.einsum("khc,hc->kh", Wr, a_d).astype(np.float32)
        ws[f"wstack{l}"] = (Wr.transpose(1, 0, 2).reshape(H * ci, -1) / H
                            ).astype(np.float32)

    t_l1 = np.zeros((NPAD, ROWP[0]), np.float32)
    ls1 = xin @ ws["ws1"]
    ld1_full = xin @ ws["wd1"]
    for c in range(NCORES):
        r0 = c * SPAD
        t_l1[r0:r0 + SHARD, 0:4] = xin[c * SHARD:(c + 1) * SHARD]
        t_l1[r0:r0 + SHARD, 4:8] = ls1[c * SHARD:(c + 1) * SHARD]

    def wrap16(flat):
        v = flat.astype(np.int32).astype(np.uint16).astype(np.int16)
        w = v.reshape(-1, 16).T
        return np.tile(w, (8, 1)).astype(np.int16)

    masks = _dropout_masks()
    iota = np.tile(np.arange(P, dtype=np.float32), (P, 1))
    in_maps = []
    for c in range(NCORES):
        src_g, dst_gl, dst_loc = graphs[c]
        flat_src = src_g.T.reshape(-1).astype(np.int64)
        flat_dst = dst_gl.T.reshape(-1).astype(np.int64)
        hi = flat_src >= IBASE
        srcA16 = wrap16(np.where(hi, 0, flat_src))
        srcB16 = wrap16(np.where(hi, flat_src - IBASE, 0))
        selB_np = hi.astype(np.uint8).reshape(-1, P).T.copy()
        dst16 = wrap16(flat_dst)
        ld1 = np.zeros((SPAD + P, LDROW), np.float32)
        ld1[:SHARD, 0:4] = ld1_full[c * SHARD:(c + 1) * SHARD]
        ld1[SPAD] = NEG
        m = {
            "t_l1": t_l1, "ld1": ld1, "src_gA": srcA16, "src_gB": srcB16,
            "selB": selB_np, "dst_gl": dst16,
            "dst_loc": dst_loc, "iota": iota,
            "ws2": ws["ws2"], "wd2": ws["wd2"],
            "ws3": ws["ws3"], "wd3": ws["wd3"],
            "a1e": np.concatenate(
                [np.asarray(inputs["A1"], np.float32),
                 np.asarray(inputs["ab1"], np.float32)[None, :]], axis=0),
            "a2": np.asarray(inputs["A2"], np.float32),
            "a3": np.asarray(inputs["A3"], np.float32),
            "ab2r": np.asarray(inputs["ab2"], np.float32)[None, :],
            "ab3": np.asarray(inputs["ab3"], np.float32)[None, :],
            "l1g": np.asarray(inputs["l1g"], np.float32)[None, :],
            "l1b": np.asarray(inputs["l1b"], np.float32)[None, :],
            "l2g": np.asarray(inputs["l2g"], np.float32)[None, :],
            "l2b": np.asarray(inputs["l2b"], np.float32)[None, :],
        }
        for l, (ci, co) in enumerate(LAYERS, start=1):
            m[f"wstack{l}"] = ws[f"wstack{l}"]
            m[f"g{l}"] = np.asarray(inputs[f"g{l}"], np.float32)[:, None]
            m[f"be{l}"] = np.asarray(inputs[f"be{l}"], np.float32)[:, None]
            msk = np.zeros((co, SPAD), np.float32)
            msk[:, :SHARD] = masks[l - 1][c * SHARD:(c + 1) * SHARD].T
            m[f"mask{l}"] = msk
        in_maps.append(m)
    return in_maps


def kernel(**inputs):
    import tprof
    tprof.install()
    from concourse.bass_utils import run_bass_kernel_spmd

    graphs, K_max, ncols = _prep_graph(np.asarray(inputs["edge_index"]))
    ck = (K_max, ncols)
    if _CACHE.get("key") != ck:
        _CACHE["nc"] = _build_nc(K_max, ncols)
        _CACHE["key"] = ck
    nc = _CACHE["nc"]

    in_maps = _host_inputs(inputs, graphs)
    res = run_bass_kernel_spmd(nc, in_maps, core_ids=list(range(NCORES)),
                               trace=False)
    t = np.concatenate([res.results[c]["tout"][:SHARD, 0] for c in range(NCORES)])

    x = np.asarray(inputs["x"], np.float32)
    cond = (x[:, 2] == 1.0) & (x[:, 5] == 0.0)
    idx = np.zeros(N, np.int64)
    nz = np.nonzero(cond)[0]
    idx[:len(nz)] = nz
    logits = t[idx].astype(np.float32).reshape(1, N)
    mx = logits.max(axis=1, keepdims=True)
    ee = np.exp(logits - mx, dtype=np.float32)
    prob = (ee / ee.sum(axis=1, keepdims=True)).astype(np.float32)
    return (logits, prob)
